# revision 26
# baseline (speedup 1.0000x reference)
# Batched OMP (K=5) dictionary-learning kernel for Trainium2, data-parallel
# over 8 NeuronCores (one image b per core; M=4096 signals/core).
#
# Per-core algorithm (tiles of 128 signals, one signal per partition):
#   per iteration k:
#     h = X^T Dn - recon_k @ Dn (PE: h_bar matmul + negated diag-scaled atom
#                                transposes, all accumulated in PSUM)
#     m, idx = argmax |h|       (DVE: abs-max reduce + max_index on [m,-m])
#     a_k = DnT[idx]            (indirect DMA row gather, [P,1] offsets)
#     Gcol/hsel dots            (DVE scalar_tensor_tensor with accum)
#     Cholesky update + solve   (batched vector ops; Pool takes the MACs)
#   outputs: support, coeffs (=gamma), z_dl_ste = x + (recon - x), loss.
import numpy as np
from contextlib import ExitStack

import concourse.bass as bass
import concourse.bacc as bacc
import concourse.tile as tile
from concourse import mybir
from concourse.bass_utils import run_bass_kernel_spmd
from concourse.masks import make_identity

F32 = mybir.dt.float32
U32 = mybir.dt.uint32
AX = mybir.AxisListType
OP = mybir.AluOpType

B, C, H, W = 8, 64, 64, 64
N = 512
K = 5
NCORES = 8
P = 128
EPS = 1e-10
SELGRP = 4  # tiles per abs-max/mn8 group; must stay < hps bufs to avoid cycles


def emit_omp(ctx: ExitStack, tc: tile.TileContext, io: dict, T: int, nstream: int, kmax: int = K, fin: bool = True, selgrp: int = SELGRP, dots_pool_frac: int = 0):
    """Emit the per-core OMP kernel. T = number of 128-signal tiles."""
    nc = tc.nc
    M = T * P
    assert T % nstream == 0
    ts_per_s = T // nstream

    const = ctx.enter_context(tc.tile_pool(name="const", bufs=1))
    state = ctx.enter_context(tc.tile_pool(name="state", bufs=1))
    dgp = ctx.enter_context(tc.tile_pool(name="dgp", bufs=8))
    rtp = ctx.enter_context(tc.tile_pool(name="rtp", bufs=3))
    finp = ctx.enter_context(tc.tile_pool(name="finp", bufs=2))
    dotp = ctx.enter_context(tc.tile_pool(name="dotp", bufs=4))
    hps = ctx.enter_context(tc.tile_pool(name="hps", bufs=6, space="PSUM"))
    smps = ctx.enter_context(tc.tile_pool(name="smps", bufs=2, space="PSUM"))

    # ---- constants / inputs in SBUF ----
    x_all = const.tile([C, M], F32)          # signals, c-major (== z_e[b])
    dn_sb = const.tile([C, N], F32)          # normalized dictionary
    ident = const.tile([P, P], F32)
    ones64 = const.tile([C, 1], F32)
    nc.sync.dma_start(x_all[:], io["x"])
    nc.sync.dma_start(dn_sb[:], io["dn"])
    make_identity(nc, ident[:])
    nc.vector.memset(ones64[:], 1.0)

    # ---- persistent state ----
    A = state.tile([P, K, T, C], F32)        # gathered atoms per selection
    Gcol = state.tile([P, K, T], F32)
    equ = state.tile([P, T], U32)
    Lp = state.tile([P, 4, K, T], F32)       # L rows 1..4 (row r -> Lp[r-1])
    gam = state.tile([P, K, T], F32)
    ngam = state.tile([P, K, T], F32)        # -gamma
    yv = state.tile([P, K, T], F32)
    m_sb = state.tile([P, T], F32)
    mn8 = state.tile([P, T, 8], F32)
    idx8 = state.tile([P, T, 8], U32)
    idxs = state.tile([P, K, T], U32)
    t1 = state.tile([P, T], F32)
    t2 = state.tile([P, T], F32)
    sacc = state.tile([P, T], F32)
    rdiag = state.tile([P, 4, T], F32)
    dotscr = state.tile([P, 2, C], F32)      # gpsimd dot dummy-outs
    ssep = state.tile([C, T], F32)
    ssetot = state.tile([C, 1], F32)
    sse_sb = state.tile([1, 1], F32)

    def tsl(t):
        return slice(t * P, (t + 1) * P)

    def ts_copy(out, in_):
        nc.vector.tensor_scalar(out, in_, 1.0, None, op0=OP.mult)

    h_of_tile = {}

    def emit_select(k, t):
        """PE-recompute h for tile t at iteration k, then abs-max reduce."""
        h = hps.tile([P, N], F32, tag="h")
        h_of_tile[t] = h
        if k == 1:
            nc.tensor.matmul(out=h[:], lhsT=x_all[:, tsl(t)], rhs=dn_sb[:],
                             start=True, stop=True)
        else:
            # residT = x - recon accumulated on PE: scaled-atom transposes
            # (lhsT=A_j*(-gam_j), rhs=identity) plus an identity-matmul +x.
            rT = smps.tile([P, P], F32, tag="sm")
            for j in range(k - 1):
                asc = dgp.tile([P, C], F32)
                nc.scalar.mul(asc[:], A[:, j, t, :], ngam[:, j, t:t + 1])
                nc.tensor.matmul(out=rT[:C, :], lhsT=asc[:], rhs=ident[:],
                                 start=(j == 0), stop=False)
            nc.tensor.matmul(out=rT[:C, :], lhsT=ident[:C, :C],
                             rhs=x_all[:, tsl(t)], start=False, stop=True)
            rT_sb = rtp.tile([C, P], F32)
            nc.scalar.copy(rT_sb[:], rT[:C, :])
            nc.tensor.matmul(out=h[:], lhsT=rT_sb[:], rhs=dn_sb[:],
                             start=True, stop=True)
        nc.vector.tensor_reduce(m_sb[:, t:t + 1], h[:], axis=AX.X,
                                op=OP.max, apply_absolute_value=True)

    def emit_mn8(tiles):
        g0, g1 = tiles[0], tiles[-1] + 1
        ng = g1 - g0
        msrc = m_sb[:, g0:g1].rearrange("p (t one) -> p t one", one=1)
        nc.vector.tensor_scalar(mn8[:, g0:g1, 0:8:2],
                                msrc.to_broadcast([P, ng, 4]),
                                1.0, None, op0=OP.mult)
        nc.vector.tensor_scalar(mn8[:, g0:g1, 1:8:2],
                                msrc.to_broadcast([P, ng, 4]),
                                -1.0, None, op0=OP.mult)

    def emit_index(k, t):
        h = h_of_tile.pop(t)
        nc.vector.max_index(idx8[:, t, :], mn8[:, t, :], h[:])

    def emit_gather(k, t):
        """Per-tile atom gather by selected index ([P,1] offsets only —
        hardware does not honor multi-column offset APs)."""
        kk = k - 1
        nc.gpsimd.indirect_dma_start(
            out=A[:, kk, t, :], out_offset=None,
            in_=io["dnt"],
            in_offset=bass.IndirectOffsetOnAxis(ap=idxs[:, kk, t:t + 1],
                                                axis=0),
        )

    def emit_dots(k, t):
        """Gram-column dot products (DVE STT-accum, optionally
        offloading the multiply of some dots to Pool)."""
        kk = k - 1
        for j in range(kk):
            if dots_pool_frac and (t % dots_pool_frac == 0):
                pr = dotp.tile([P, C], F32)
                nc.gpsimd.tensor_tensor(out=pr[:], in0=A[:, j, t, :],
                                        in1=A[:, kk, t, :], op=OP.mult)
                nc.vector.tensor_reduce(Gcol[:, j, t:t + 1], pr[:],
                                        axis=AX.X, op=OP.add)
            else:
                nc.vector.scalar_tensor_tensor(
                    out=dotscr[:, t % 2, :], in0=A[:, j, t, :], scalar=1.0,
                    in1=A[:, kk, t, :], op0=OP.mult, op1=OP.mult,
                    accum_out=Gcol[:, j, t:t + 1])

    def TT(out, a, b, op):
        nc.vector.tensor_tensor(out=out, in0=a, in1=b, op=op)

    def PT(out, a, b, op):
        nc.vector.tensor_tensor(out=out, in0=a, in1=b, op=op)

    def emit_selval(k, S):
        """y_k from the selected h value: h_k[i_k] = +-m, sign from whether
        the +m slot of max_index won; forward solve collapses to
        y_k = selval / L_kk (identical to the reference up to fp rounding)."""
        r = k - 1
        TT(equ[:, S], idxs[:, r, S], idx8[:, S, 0], OP.is_equal)
        nc.vector.tensor_copy(t2[:, S], equ[:, S])
        nc.vector.tensor_scalar(t2[:, S], t2[:, S], 2.0, -1.0,
                                op0=OP.mult, op1=OP.add)
        TT(yv[:, r, S], t2[:, S], m_sb[:, S], OP.mult)

    def emit_solve(k, S):
        """Batched solve over tile slice S (solve state in selection order)."""
        r = k - 1  # new row index (0-based) of L
        if k == 1:
            ts_copy(gam[:, 0, S], yv[:, 0, S])
        else:
            # --- w row: w_i for i=0..r-1 stored into Lp[r-1][i] ---
            ts_copy(Lp[:, r - 1, 0, S], Gcol[:, 0, S])
            for i in range(1, r):
                ts_copy(t1[:, S], Gcol[:, i, S])
                for q in range(i):
                    PT(t2[:, S], Lp[:, i - 1, q, S], Lp[:, r - 1, q, S], OP.mult)
                    PT(t1[:, S], t1[:, S], t2[:, S], OP.subtract)
                TT(Lp[:, r - 1, i, S], t1[:, S], rdiag[:, i - 1, S], OP.mult)
            # --- corner = sqrt(clip(1 - sum w^2, 1e-12)) ---
            PT(sacc[:, S], Lp[:, r - 1, 0, S], Lp[:, r - 1, 0, S], OP.mult)
            for i in range(1, r):
                PT(t2[:, S], Lp[:, r - 1, i, S], Lp[:, r - 1, i, S], OP.mult)
                PT(sacc[:, S], sacc[:, S], t2[:, S], OP.add)
            nc.vector.tensor_scalar(t1[:, S], sacc[:, S], -1.0, 1.0,
                                    op0=OP.mult, op1=OP.add)
            nc.vector.tensor_scalar(t1[:, S], t1[:, S], 1e-12, None, op0=OP.max)
            nc.scalar.sqrt(Lp[:, r - 1, r, S], t1[:, S])
            nc.vector.reciprocal(out=rdiag[:, r - 1, S], in_=Lp[:, r - 1, r, S])
            # --- forward solve collapsed: y_r = selval / L_rr ---
            TT(yv[:, r, S], yv[:, r, S], rdiag[:, r - 1, S], OP.mult)
            # --- backward solve L^T gam = y ---
            for i in range(k - 1, -1, -1):
                ts_copy(t1[:, S], yv[:, i, S])
                for q in range(i + 1, k):
                    PT(t2[:, S], Lp[:, q - 1, i, S], gam[:, q, S], OP.mult)
                    PT(t1[:, S], t1[:, S], t2[:, S], OP.subtract)
                if i == 0:
                    ts_copy(gam[:, 0, S], t1[:, S])
                else:
                    TT(gam[:, i, S], t1[:, S], rdiag[:, i - 1, S], OP.mult)
        if k < K:
            nc.vector.tensor_scalar(ngam[:, 0:k, S], gam[:, 0:k, S], -1.0,
                                    None, op0=OP.mult)

    # ---- main iteration loop, streams interleaved for pipelining ----
    for k in range(1, kmax + 1):
        for s in range(nstream):
            tiles = list(range(s * ts_per_s, (s + 1) * ts_per_s))
            S = slice(s * ts_per_s, (s + 1) * ts_per_s)
            for g0 in range(0, len(tiles), selgrp):
                grp = tiles[g0:g0 + selgrp]
                for t in grp:
                    emit_select(k, t)
                emit_mn8(grp)
                for t in grp:
                    emit_index(k, t)
            nc.vector.tensor_reduce(idxs[:, k - 1, S], idx8[:, S, 0:2],
                                    axis=AX.X, op=OP.min)
            emit_selval(k, S)
            for t in tiles:
                emit_gather(k, t)
        for s in range(nstream):
            tiles = list(range(s * ts_per_s, (s + 1) * ts_per_s))
            S = slice(s * ts_per_s, (s + 1) * ts_per_s)
            for t in tiles:
                emit_dots(k, t)
            emit_solve(k, S)

    # ---- finalize: recon, outputs, loss ----
    for t in range(T if fin else 0):
        recT = smps.tile([P, P], F32, tag="sm")
        for j in range(K):
            asc = dgp.tile([P, C], F32)
            nc.scalar.mul(asc[:], A[:, j, t, :], gam[:, j, t:t + 1])
            nc.tensor.matmul(out=recT[:C, :], lhsT=asc[:], rhs=ident[:],
                             start=(j == 0), stop=(j == K - 1))
        errT = finp.tile([C, P], F32, tag="err")
        steT = finp.tile([C, P], F32, tag="ste")
        sqT = finp.tile([C, P], F32, tag="sq")
        TT(errT[:], recT[:C, :], x_all[:, tsl(t)], OP.subtract)
        nc.vector.scalar_tensor_tensor(
            out=sqT[:], in0=errT[:], scalar=1.0, in1=errT[:],
            op0=OP.mult, op1=OP.mult, accum_out=ssep[:, t:t + 1])
        TT(steT[:], x_all[:, tsl(t)], errT[:], OP.add)
        nc.sync.dma_start(io["zdl"][:, tsl(t)], steT[:])

    # support / coeffs: SBUF [P, K, T] -> DRAM [M, K] with m = t*128+p
    sup_ap = bass.AP(tensor=io["sup"].tensor, offset=0,
                     ap=[[K, P], [1, K], [K * P, T]])
    coe_ap = bass.AP(tensor=io["coe"].tensor, offset=0,
                     ap=[[K, P], [1, K], [K * P, T]])
    nc.sync.dma_start(sup_ap, idxs[:].bitcast(mybir.dt.int32))
    nc.sync.dma_start(coe_ap, gam[:])

    if not fin:
        nc.vector.memset(ssep[:], 0.0)
        zz = finp.tile([C, P], F32, tag="ste")
        nc.vector.memset(zz[:], 0.0)
        for t in range(T):
            nc.sync.dma_start(io["zdl"][:, tsl(t)], zz[:])
    # loss partial: sum over everything via ones-matmul
    nc.vector.tensor_reduce(ssetot[:], ssep[:], axis=AX.X, op=OP.add)
    lps = smps.tile([P, P], F32, tag="sm")
    nc.tensor.matmul(out=lps[:1, :1], lhsT=ssetot[:], rhs=ones64[:],
                     start=True, stop=True)
    nc.scalar.copy(sse_sb[:], lps[:1, :1])
    nc.sync.dma_start(io["sse"], sse_sb[:])


def build_nc(T: int = H * W // P, nstream: int = 2, kmax: int = K, fin: bool = True, selgrp: int = SELGRP, dots_pool_frac: int = 0):
    nc = bacc.Bacc("TRN2", target_bir_lowering=False, debug=False,
                   num_devices=NCORES)
    M = T * P
    io = dict(
        x=nc.dram_tensor("x", [C, M], F32, kind="ExternalInput").ap(),
        dn=nc.dram_tensor("dn", [C, N], F32, kind="ExternalInput").ap(),
        dnt=nc.dram_tensor("dnt", [N, C], F32, kind="ExternalInput").ap(),
        zdl=nc.dram_tensor("zdl", [C, M], F32, kind="ExternalOutput").ap(),
        sup=nc.dram_tensor("sup", [M, K], mybir.dt.int32,
                           kind="ExternalOutput").ap(),
        coe=nc.dram_tensor("coe", [M, K], F32, kind="ExternalOutput").ap(),
        sse=nc.dram_tensor("sse", [1, 1], F32, kind="ExternalOutput").ap(),
    )
    with tile.TileContext(nc) as tc:
        with ExitStack() as ctx:
            emit_omp(ctx, tc, io, T, nstream, kmax=kmax, fin=fin, selgrp=selgrp, dots_pool_frac=dots_pool_frac)
    nc.compile()
    return nc


_NC_CACHE: dict = {}


def _get_nc():
    if "nc" not in _NC_CACHE:
        _NC_CACHE["nc"] = build_nc()
    return _NC_CACHE["nc"]


def host_prep(dictionary: np.ndarray):
    """Normalize dictionary columns exactly like the reference (fp32)."""
    d = dictionary.astype(np.float32)
    norms = np.sqrt(np.sum(d * d, axis=0, dtype=np.float32),
                    dtype=np.float32).astype(np.float32)
    dn = d / np.maximum(norms, np.float32(EPS))[None, :]
    return np.ascontiguousarray(dn.astype(np.float32))


def kernel(z_e: np.ndarray, dictionary: np.ndarray, trace: bool = False):
    z_e = np.asarray(z_e, dtype=np.float32)
    dn = host_prep(np.asarray(dictionary))
    dnt = np.ascontiguousarray(dn.T)
    nc = _get_nc()
    in_maps = [
        {"x": np.ascontiguousarray(z_e[b].reshape(C, H * W)),
         "dn": dn, "dnt": dnt}
        for b in range(B)
    ]
    res = run_bass_kernel_spmd(nc, in_maps, core_ids=list(range(NCORES)),
                               trace=trace)
    outs = res.results
    z_dl = np.stack([outs[b]["zdl"].reshape(C, H, W) for b in range(B)])
    support = np.stack([outs[b]["sup"].reshape(H, W, K) for b in range(B)])
    coeffs = np.stack([outs[b]["coe"].reshape(H, W, K) for b in range(B)])
    sse = np.sum([outs[b]["sse"][0, 0] for b in range(B)], dtype=np.float32)
    nelem = np.float32(B * C * H * W)
    mse = np.float32(sse / nelem)
    loss = np.float32(mse + 0.25 * mse)
    if trace:
        return (z_dl, loss, support, coeffs), res
    return z_dl, loss, support, coeffs


# revision 27
# speedup vs baseline: 1.0052x; 1.0052x over previous
# Batched OMP (K=5) dictionary-learning kernel for Trainium2, data-parallel
# over 8 NeuronCores (one image b per core; M=4096 signals/core).
#
# Per-core algorithm (tiles of 128 signals, one signal per partition):
#   per iteration k:
#     h = X^T Dn - recon_k @ Dn (PE: h_bar matmul + negated diag-scaled atom
#                                transposes, all accumulated in PSUM)
#     m, idx = argmax |h|       (DVE: abs-max reduce + max_index on [m,-m])
#     a_k = DnT[idx]            (indirect DMA row gather, [P,1] offsets)
#     Gcol/hsel dots            (DVE scalar_tensor_tensor with accum)
#     Cholesky update + solve   (batched vector ops; Pool takes the MACs)
#   outputs: support, coeffs (=gamma), z_dl_ste = x + (recon - x), loss.
import numpy as np
from contextlib import ExitStack

import concourse.bass as bass
import concourse.bacc as bacc
import concourse.tile as tile
from concourse import mybir
from concourse.bass_utils import run_bass_kernel_spmd
from concourse.masks import make_identity

F32 = mybir.dt.float32
U32 = mybir.dt.uint32
AX = mybir.AxisListType
OP = mybir.AluOpType

B, C, H, W = 8, 64, 64, 64
N = 512
K = 5
NCORES = 8
P = 128
EPS = 1e-10
SELGRP = 2  # tiles per abs-max/mn8 group; must stay < hps bufs to avoid cycles


def emit_omp(ctx: ExitStack, tc: tile.TileContext, io: dict, T: int, nstream: int, kmax: int = K, fin: bool = True, selgrp: int = SELGRP, dots_pool_frac: int = 0):
    """Emit the per-core OMP kernel. T = number of 128-signal tiles."""
    nc = tc.nc
    M = T * P
    assert T % nstream == 0
    ts_per_s = T // nstream

    const = ctx.enter_context(tc.tile_pool(name="const", bufs=1))
    state = ctx.enter_context(tc.tile_pool(name="state", bufs=1))
    dgp = ctx.enter_context(tc.tile_pool(name="dgp", bufs=8))
    rtp = ctx.enter_context(tc.tile_pool(name="rtp", bufs=3))
    finp = ctx.enter_context(tc.tile_pool(name="finp", bufs=2))
    dotp = ctx.enter_context(tc.tile_pool(name="dotp", bufs=4))
    hps = ctx.enter_context(tc.tile_pool(name="hps", bufs=6, space="PSUM"))
    smps = ctx.enter_context(tc.tile_pool(name="smps", bufs=2, space="PSUM"))

    # ---- constants / inputs in SBUF ----
    x_all = const.tile([C, M], F32)          # signals, c-major (== z_e[b])
    dn_sb = const.tile([C, N], F32)          # normalized dictionary
    ident = const.tile([P, P], F32)
    ones64 = const.tile([C, 1], F32)
    nc.sync.dma_start(x_all[:], io["x"])
    nc.sync.dma_start(dn_sb[:], io["dn"])
    make_identity(nc, ident[:])
    nc.vector.memset(ones64[:], 1.0)

    # ---- persistent state ----
    A = state.tile([P, K, T, C], F32)        # gathered atoms per selection
    Gcol = state.tile([P, K, T], F32)
    equ = state.tile([P, T], U32)
    Lp = state.tile([P, 4, K, T], F32)       # L rows 1..4 (row r -> Lp[r-1])
    gam = state.tile([P, K, T], F32)
    ngam = state.tile([P, K, T], F32)        # -gamma
    yv = state.tile([P, K, T], F32)
    m_sb = state.tile([P, T], F32)
    mn8 = state.tile([P, T, 8], F32)
    idx8 = state.tile([P, T, 8], U32)
    idxs = state.tile([P, K, T], U32)
    t1 = state.tile([P, T], F32)
    t2 = state.tile([P, T], F32)
    sacc = state.tile([P, T], F32)
    rdiag = state.tile([P, 4, T], F32)
    dotscr = state.tile([P, 2, C], F32)      # gpsimd dot dummy-outs
    ssep = state.tile([C, T], F32)
    ssetot = state.tile([C, 1], F32)
    sse_sb = state.tile([1, 1], F32)

    def tsl(t):
        return slice(t * P, (t + 1) * P)

    def ts_copy(out, in_):
        nc.vector.tensor_scalar(out, in_, 1.0, None, op0=OP.mult)

    h_of_tile = {}

    def emit_select(k, t):
        """PE-recompute h for tile t at iteration k, then abs-max reduce."""
        h = hps.tile([P, N], F32, tag="h")
        h_of_tile[t] = h
        if k == 1:
            nc.tensor.matmul(out=h[:], lhsT=x_all[:, tsl(t)], rhs=dn_sb[:],
                             start=True, stop=True)
        else:
            # residT = x - recon accumulated on PE: scaled-atom transposes
            # (lhsT=A_j*(-gam_j), rhs=identity) plus an identity-matmul +x.
            rT = smps.tile([P, P], F32, tag="sm")
            for j in range(k - 1):
                asc = dgp.tile([P, C], F32)
                nc.scalar.mul(asc[:], A[:, j, t, :], ngam[:, j, t:t + 1])
                nc.tensor.matmul(out=rT[:C, :], lhsT=asc[:], rhs=ident[:],
                                 start=(j == 0), stop=False)
            nc.tensor.matmul(out=rT[:C, :], lhsT=ident[:C, :C],
                             rhs=x_all[:, tsl(t)], start=False, stop=True)
            rT_sb = rtp.tile([C, P], F32)
            nc.scalar.copy(rT_sb[:], rT[:C, :])
            nc.tensor.matmul(out=h[:], lhsT=rT_sb[:], rhs=dn_sb[:],
                             start=True, stop=True)
        nc.vector.tensor_reduce(m_sb[:, t:t + 1], h[:], axis=AX.X,
                                op=OP.max, apply_absolute_value=True)

    def emit_mn8(tiles):
        g0, g1 = tiles[0], tiles[-1] + 1
        ng = g1 - g0
        msrc = m_sb[:, g0:g1].rearrange("p (t one) -> p t one", one=1)
        nc.vector.tensor_scalar(mn8[:, g0:g1, 0:8:2],
                                msrc.to_broadcast([P, ng, 4]),
                                1.0, None, op0=OP.mult)
        nc.vector.tensor_scalar(mn8[:, g0:g1, 1:8:2],
                                msrc.to_broadcast([P, ng, 4]),
                                -1.0, None, op0=OP.mult)

    def emit_index(k, t):
        h = h_of_tile.pop(t)
        nc.vector.max_index(idx8[:, t, :], mn8[:, t, :], h[:])

    def emit_gather(k, t):
        """Per-tile atom gather by selected index ([P,1] offsets only —
        hardware does not honor multi-column offset APs)."""
        kk = k - 1
        nc.gpsimd.indirect_dma_start(
            out=A[:, kk, t, :], out_offset=None,
            in_=io["dnt"],
            in_offset=bass.IndirectOffsetOnAxis(ap=idxs[:, kk, t:t + 1],
                                                axis=0),
        )

    def emit_dots(k, t):
        """Gram-column dot products (DVE STT-accum, optionally
        offloading the multiply of some dots to Pool)."""
        kk = k - 1
        for j in range(kk):
            if dots_pool_frac and (t % dots_pool_frac == 0):
                pr = dotp.tile([P, C], F32)
                nc.gpsimd.tensor_tensor(out=pr[:], in0=A[:, j, t, :],
                                        in1=A[:, kk, t, :], op=OP.mult)
                nc.vector.tensor_reduce(Gcol[:, j, t:t + 1], pr[:],
                                        axis=AX.X, op=OP.add)
            else:
                nc.vector.scalar_tensor_tensor(
                    out=dotscr[:, t % 2, :], in0=A[:, j, t, :], scalar=1.0,
                    in1=A[:, kk, t, :], op0=OP.mult, op1=OP.mult,
                    accum_out=Gcol[:, j, t:t + 1])

    def TT(out, a, b, op):
        nc.vector.tensor_tensor(out=out, in0=a, in1=b, op=op)

    def PT(out, a, b, op):
        nc.vector.tensor_tensor(out=out, in0=a, in1=b, op=op)

    def emit_selval(k, S):
        """y_k from the selected h value: h_k[i_k] = +-m, sign from whether
        the +m slot of max_index won; forward solve collapses to
        y_k = selval / L_kk (identical to the reference up to fp rounding)."""
        r = k - 1
        TT(equ[:, S], idxs[:, r, S], idx8[:, S, 0], OP.is_equal)
        nc.vector.tensor_copy(t2[:, S], equ[:, S])
        nc.vector.tensor_scalar(t2[:, S], t2[:, S], 2.0, -1.0,
                                op0=OP.mult, op1=OP.add)
        TT(yv[:, r, S], t2[:, S], m_sb[:, S], OP.mult)

    def emit_solve(k, S):
        """Batched solve over tile slice S (solve state in selection order)."""
        r = k - 1  # new row index (0-based) of L
        if k == 1:
            ts_copy(gam[:, 0, S], yv[:, 0, S])
        else:
            # --- w row: w_i for i=0..r-1 stored into Lp[r-1][i] ---
            ts_copy(Lp[:, r - 1, 0, S], Gcol[:, 0, S])
            for i in range(1, r):
                ts_copy(t1[:, S], Gcol[:, i, S])
                for q in range(i):
                    PT(t2[:, S], Lp[:, i - 1, q, S], Lp[:, r - 1, q, S], OP.mult)
                    PT(t1[:, S], t1[:, S], t2[:, S], OP.subtract)
                TT(Lp[:, r - 1, i, S], t1[:, S], rdiag[:, i - 1, S], OP.mult)
            # --- corner = sqrt(clip(1 - sum w^2, 1e-12)) ---
            PT(sacc[:, S], Lp[:, r - 1, 0, S], Lp[:, r - 1, 0, S], OP.mult)
            for i in range(1, r):
                PT(t2[:, S], Lp[:, r - 1, i, S], Lp[:, r - 1, i, S], OP.mult)
                PT(sacc[:, S], sacc[:, S], t2[:, S], OP.add)
            nc.vector.tensor_scalar(t1[:, S], sacc[:, S], -1.0, 1.0,
                                    op0=OP.mult, op1=OP.add)
            nc.vector.tensor_scalar(t1[:, S], t1[:, S], 1e-12, None, op0=OP.max)
            nc.scalar.sqrt(Lp[:, r - 1, r, S], t1[:, S])
            nc.vector.reciprocal(out=rdiag[:, r - 1, S], in_=Lp[:, r - 1, r, S])
            # --- forward solve collapsed: y_r = selval / L_rr ---
            TT(yv[:, r, S], yv[:, r, S], rdiag[:, r - 1, S], OP.mult)
            # --- backward solve L^T gam = y ---
            for i in range(k - 1, -1, -1):
                ts_copy(t1[:, S], yv[:, i, S])
                for q in range(i + 1, k):
                    PT(t2[:, S], Lp[:, q - 1, i, S], gam[:, q, S], OP.mult)
                    PT(t1[:, S], t1[:, S], t2[:, S], OP.subtract)
                if i == 0:
                    ts_copy(gam[:, 0, S], t1[:, S])
                else:
                    TT(gam[:, i, S], t1[:, S], rdiag[:, i - 1, S], OP.mult)
        if k < K:
            nc.vector.tensor_scalar(ngam[:, 0:k, S], gam[:, 0:k, S], -1.0,
                                    None, op0=OP.mult)

    # ---- main iteration loop, streams interleaved for pipelining ----
    for k in range(1, kmax + 1):
        for s in range(nstream):
            tiles = list(range(s * ts_per_s, (s + 1) * ts_per_s))
            S = slice(s * ts_per_s, (s + 1) * ts_per_s)
            for g0 in range(0, len(tiles), selgrp):
                grp = tiles[g0:g0 + selgrp]
                for t in grp:
                    emit_select(k, t)
                emit_mn8(grp)
                for t in grp:
                    emit_index(k, t)
            nc.vector.tensor_reduce(idxs[:, k - 1, S], idx8[:, S, 0:2],
                                    axis=AX.X, op=OP.min)
            emit_selval(k, S)
            for t in tiles:
                emit_gather(k, t)
        for s in range(nstream):
            tiles = list(range(s * ts_per_s, (s + 1) * ts_per_s))
            S = slice(s * ts_per_s, (s + 1) * ts_per_s)
            for t in tiles:
                emit_dots(k, t)
            emit_solve(k, S)

    # ---- finalize: recon, outputs, loss ----
    for t in range(T if fin else 0):
        recT = smps.tile([P, P], F32, tag="sm")
        for j in range(K):
            asc = dgp.tile([P, C], F32)
            nc.scalar.mul(asc[:], A[:, j, t, :], gam[:, j, t:t + 1])
            nc.tensor.matmul(out=recT[:C, :], lhsT=asc[:], rhs=ident[:],
                             start=(j == 0), stop=(j == K - 1))
        errT = finp.tile([C, P], F32, tag="err")
        steT = finp.tile([C, P], F32, tag="ste")
        sqT = finp.tile([C, P], F32, tag="sq")
        TT(errT[:], recT[:C, :], x_all[:, tsl(t)], OP.subtract)
        nc.vector.scalar_tensor_tensor(
            out=sqT[:], in0=errT[:], scalar=1.0, in1=errT[:],
            op0=OP.mult, op1=OP.mult, accum_out=ssep[:, t:t + 1])
        TT(steT[:], x_all[:, tsl(t)], errT[:], OP.add)
        nc.sync.dma_start(io["zdl"][:, tsl(t)], steT[:])

    # support / coeffs: SBUF [P, K, T] -> DRAM [M, K] with m = t*128+p
    sup_ap = bass.AP(tensor=io["sup"].tensor, offset=0,
                     ap=[[K, P], [1, K], [K * P, T]])
    coe_ap = bass.AP(tensor=io["coe"].tensor, offset=0,
                     ap=[[K, P], [1, K], [K * P, T]])
    nc.sync.dma_start(sup_ap, idxs[:].bitcast(mybir.dt.int32))
    nc.sync.dma_start(coe_ap, gam[:])

    if not fin:
        nc.vector.memset(ssep[:], 0.0)
        zz = finp.tile([C, P], F32, tag="ste")
        nc.vector.memset(zz[:], 0.0)
        for t in range(T):
            nc.sync.dma_start(io["zdl"][:, tsl(t)], zz[:])
    # loss partial: sum over everything via ones-matmul
    nc.vector.tensor_reduce(ssetot[:], ssep[:], axis=AX.X, op=OP.add)
    lps = smps.tile([P, P], F32, tag="sm")
    nc.tensor.matmul(out=lps[:1, :1], lhsT=ssetot[:], rhs=ones64[:],
                     start=True, stop=True)
    nc.scalar.copy(sse_sb[:], lps[:1, :1])
    nc.sync.dma_start(io["sse"], sse_sb[:])


def build_nc(T: int = H * W // P, nstream: int = 2, kmax: int = K, fin: bool = True, selgrp: int = SELGRP, dots_pool_frac: int = 0):
    nc = bacc.Bacc("TRN2", target_bir_lowering=False, debug=False,
                   num_devices=NCORES)
    M = T * P
    io = dict(
        x=nc.dram_tensor("x", [C, M], F32, kind="ExternalInput").ap(),
        dn=nc.dram_tensor("dn", [C, N], F32, kind="ExternalInput").ap(),
        dnt=nc.dram_tensor("dnt", [N, C], F32, kind="ExternalInput").ap(),
        zdl=nc.dram_tensor("zdl", [C, M], F32, kind="ExternalOutput").ap(),
        sup=nc.dram_tensor("sup", [M, K], mybir.dt.int32,
                           kind="ExternalOutput").ap(),
        coe=nc.dram_tensor("coe", [M, K], F32, kind="ExternalOutput").ap(),
        sse=nc.dram_tensor("sse", [1, 1], F32, kind="ExternalOutput").ap(),
    )
    with tile.TileContext(nc) as tc:
        with ExitStack() as ctx:
            emit_omp(ctx, tc, io, T, nstream, kmax=kmax, fin=fin, selgrp=selgrp, dots_pool_frac=dots_pool_frac)
    nc.compile()
    return nc


_NC_CACHE: dict = {}


def _get_nc():
    if "nc" not in _NC_CACHE:
        _NC_CACHE["nc"] = build_nc()
    return _NC_CACHE["nc"]


def host_prep(dictionary: np.ndarray):
    """Normalize dictionary columns exactly like the reference (fp32)."""
    d = dictionary.astype(np.float32)
    norms = np.sqrt(np.sum(d * d, axis=0, dtype=np.float32),
                    dtype=np.float32).astype(np.float32)
    dn = d / np.maximum(norms, np.float32(EPS))[None, :]
    return np.ascontiguousarray(dn.astype(np.float32))


def kernel(z_e: np.ndarray, dictionary: np.ndarray, trace: bool = False):
    z_e = np.asarray(z_e, dtype=np.float32)
    dn = host_prep(np.asarray(dictionary))
    dnt = np.ascontiguousarray(dn.T)
    nc = _get_nc()
    in_maps = [
        {"x": np.ascontiguousarray(z_e[b].reshape(C, H * W)),
         "dn": dn, "dnt": dnt}
        for b in range(B)
    ]
    res = run_bass_kernel_spmd(nc, in_maps, core_ids=list(range(NCORES)),
                               trace=trace)
    outs = res.results
    z_dl = np.stack([outs[b]["zdl"].reshape(C, H, W) for b in range(B)])
    support = np.stack([outs[b]["sup"].reshape(H, W, K) for b in range(B)])
    coeffs = np.stack([outs[b]["coe"].reshape(H, W, K) for b in range(B)])
    sse = np.sum([outs[b]["sse"][0, 0] for b in range(B)], dtype=np.float32)
    nelem = np.float32(B * C * H * W)
    mse = np.float32(sse / nelem)
    loss = np.float32(mse + 0.25 * mse)
    if trace:
        return (z_dl, loss, support, coeffs), res
    return z_dl, loss, support, coeffs


# revision 28
# speedup vs baseline: 1.0745x; 1.0690x over previous
# Batched OMP (K=5) dictionary-learning kernel for Trainium2, data-parallel
# over 8 NeuronCores (one image b per core; M=4096 signals/core).
#
# Per-core algorithm (tiles of 128 signals, one signal per partition):
#   per iteration k:
#     h = X^T Dn - recon_k @ Dn (PE: h_bar matmul + negated diag-scaled atom
#                                transposes, all accumulated in PSUM)
#     m, idx = argmax |h|       (DVE: abs-max reduce + max_index on [m,-m])
#     a_k = DnT[idx]            (indirect DMA row gather, [P,1] offsets)
#     Gcol/hsel dots            (DVE scalar_tensor_tensor with accum)
#     Cholesky update + solve   (batched vector ops; Pool takes the MACs)
#   outputs: support, coeffs (=gamma), z_dl_ste = x + (recon - x), loss.
import numpy as np
from contextlib import ExitStack

import concourse.bass as bass
import concourse.bacc as bacc
import concourse.tile as tile
from concourse import mybir
from concourse.bass_utils import run_bass_kernel_spmd
from concourse.masks import make_identity

F32 = mybir.dt.float32
U32 = mybir.dt.uint32
AX = mybir.AxisListType
OP = mybir.AluOpType

B, C, H, W = 8, 64, 64, 64
N = 512
K = 5
NCORES = 8
P = 128
EPS = 1e-10
SELGRP = 2  # tiles per abs-max/mn8 group; must stay < hps bufs to avoid cycles


def emit_omp(ctx: ExitStack, tc: tile.TileContext, io: dict, T: int, nstream: int, kmax: int = K, fin: bool = True, selgrp: int = SELGRP, dots_pool_frac: int = 0):
    """Emit the per-core OMP kernel. T = number of 128-signal tiles."""
    nc = tc.nc
    M = T * P
    assert T % nstream == 0
    ts_per_s = T // nstream

    const = ctx.enter_context(tc.tile_pool(name="const", bufs=1))
    state = ctx.enter_context(tc.tile_pool(name="state", bufs=1))
    dgp = ctx.enter_context(tc.tile_pool(name="dgp", bufs=8))
    rtp = ctx.enter_context(tc.tile_pool(name="rtp", bufs=3))
    finp = ctx.enter_context(tc.tile_pool(name="finp", bufs=2))
    dotp = ctx.enter_context(tc.tile_pool(name="dotp", bufs=4))
    hps = ctx.enter_context(tc.tile_pool(name="hps", bufs=6, space="PSUM"))
    smps = ctx.enter_context(tc.tile_pool(name="smps", bufs=2, space="PSUM"))

    # ---- constants / inputs in SBUF ----
    x_all = const.tile([C, M], F32)          # signals, c-major (== z_e[b])
    dn_sb = const.tile([C, N], F32)          # normalized dictionary
    ident = const.tile([P, P], F32)
    ones64 = const.tile([C, 1], F32)
    nc.sync.dma_start(x_all[:], io["x"])
    nc.sync.dma_start(dn_sb[:], io["dn"])
    make_identity(nc, ident[:])
    nc.vector.memset(ones64[:], 1.0)

    # ---- persistent state ----
    A = state.tile([P, K, T, C], F32)        # gathered atoms per selection
    Gcol = state.tile([P, K, T], F32)
    equ = state.tile([P, T], U32)
    Lp = state.tile([P, 4, K, T], F32)       # L rows 1..4 (row r -> Lp[r-1])
    gam = state.tile([P, K, T], F32)
    ngam = state.tile([P, K, T], F32)        # -gamma
    yv = state.tile([P, K, T], F32)
    m_sb = state.tile([P, T], F32)
    mn8 = state.tile([P, T, 8], F32)
    idx8 = state.tile([P, T, 8], U32)
    idxs = state.tile([P, K, T], U32)
    t1 = state.tile([P, T], F32)
    t2 = state.tile([P, T], F32)
    sacc = state.tile([P, T], F32)
    rdiag = state.tile([P, 4, T], F32)
    dotscr = state.tile([P, 2, C], F32)      # gpsimd dot dummy-outs
    ssep = state.tile([C, T], F32)
    ssetot = state.tile([C, 1], F32)
    sse_sb = state.tile([1, 1], F32)

    def tsl(t):
        return slice(t * P, (t + 1) * P)

    def ts_copy(out, in_):
        nc.vector.tensor_scalar(out, in_, 1.0, None, op0=OP.mult)

    h_of_tile = {}

    def emit_select(k, t):
        """PE-recompute h for tile t at iteration k, then abs-max reduce."""
        h = hps.tile([P, N], F32, tag="h")
        h_of_tile[t] = h
        if k == 1:
            nc.tensor.matmul(out=h[:], lhsT=x_all[:, tsl(t)], rhs=dn_sb[:],
                             start=True, stop=True)
        else:
            # residT = x - recon accumulated on PE: scaled-atom transposes
            # (lhsT=A_j*(-gam_j), rhs=identity) plus an identity-matmul +x.
            rT = smps.tile([P, P], F32, tag="sm")
            for j in range(k - 1):
                asc = dgp.tile([P, C], F32)
                nc.scalar.mul(asc[:], A[:, j, t, :], ngam[:, j, t:t + 1])
                nc.tensor.matmul(out=rT[:C, :], lhsT=asc[:], rhs=ident[:],
                                 start=(j == 0), stop=False)
            nc.tensor.matmul(out=rT[:C, :], lhsT=ident[:C, :C],
                             rhs=x_all[:, tsl(t)], start=False, stop=True)
            rT_sb = rtp.tile([C, P], F32)
            nc.scalar.copy(rT_sb[:], rT[:C, :])
            nc.tensor.matmul(out=h[:], lhsT=rT_sb[:], rhs=dn_sb[:],
                             start=True, stop=True)
        nc.vector.tensor_reduce(m_sb[:, t:t + 1], h[:], axis=AX.X,
                                op=OP.max, apply_absolute_value=True)

    def emit_mn8(tiles):
        g0, g1 = tiles[0], tiles[-1] + 1
        ng = g1 - g0
        msrc = m_sb[:, g0:g1].rearrange("p (t one) -> p t one", one=1)
        nc.vector.tensor_scalar(mn8[:, g0:g1, 0:8:2],
                                msrc.to_broadcast([P, ng, 4]),
                                1.0, None, op0=OP.mult)
        nc.vector.tensor_scalar(mn8[:, g0:g1, 1:8:2],
                                msrc.to_broadcast([P, ng, 4]),
                                -1.0, None, op0=OP.mult)

    def emit_index(k, t):
        h = h_of_tile.pop(t)
        nc.vector.max_index(idx8[:, t, :], mn8[:, t, :], h[:])

    def emit_gather(k, t):
        """Per-tile atom gather by selected index ([P,1] offsets only —
        hardware does not honor multi-column offset APs)."""
        kk = k - 1
        nc.gpsimd.indirect_dma_start(
            out=A[:, kk, t, :], out_offset=None,
            in_=io["dnt"],
            in_offset=bass.IndirectOffsetOnAxis(ap=idxs[:, kk, t:t + 1],
                                                axis=0),
        )

    def emit_dots(k, t):
        """Gram-column dot products (DVE STT-accum, optionally
        offloading the multiply of some dots to Pool)."""
        kk = k - 1
        for j in range(kk):
            if dots_pool_frac and (t % dots_pool_frac == 0):
                pr = dotp.tile([P, C], F32)
                nc.gpsimd.tensor_tensor(out=pr[:], in0=A[:, j, t, :],
                                        in1=A[:, kk, t, :], op=OP.mult)
                nc.vector.tensor_reduce(Gcol[:, j, t:t + 1], pr[:],
                                        axis=AX.X, op=OP.add)
            else:
                nc.vector.scalar_tensor_tensor(
                    out=dotscr[:, t % 2, :], in0=A[:, j, t, :], scalar=1.0,
                    in1=A[:, kk, t, :], op0=OP.mult, op1=OP.mult,
                    accum_out=Gcol[:, j, t:t + 1])

    def TT(out, a, b, op):
        nc.vector.tensor_tensor(out=out, in0=a, in1=b, op=op)

    def PT(out, a, b, op):
        nc.vector.tensor_tensor(out=out, in0=a, in1=b, op=op)

    def emit_selval(k, S):
        """y_k from the selected h value: h_k[i_k] = +-m, sign from whether
        the +m slot of max_index won; forward solve collapses to
        y_k = selval / L_kk (identical to the reference up to fp rounding)."""
        r = k - 1
        TT(equ[:, S], idxs[:, r, S], idx8[:, S, 0], OP.is_equal)
        nc.vector.tensor_copy(t2[:, S], equ[:, S])
        nc.vector.tensor_scalar(t2[:, S], t2[:, S], 2.0, -1.0,
                                op0=OP.mult, op1=OP.add)
        TT(yv[:, r, S], t2[:, S], m_sb[:, S], OP.mult)

    def emit_solve(k, S):
        """Batched solve over tile slice S (solve state in selection order)."""
        r = k - 1  # new row index (0-based) of L
        if k == 1:
            ts_copy(gam[:, 0, S], yv[:, 0, S])
        else:
            # --- w row: w_i for i=0..r-1 stored into Lp[r-1][i] ---
            ts_copy(Lp[:, r - 1, 0, S], Gcol[:, 0, S])
            for i in range(1, r):
                ts_copy(t1[:, S], Gcol[:, i, S])
                for q in range(i):
                    PT(t2[:, S], Lp[:, i - 1, q, S], Lp[:, r - 1, q, S], OP.mult)
                    PT(t1[:, S], t1[:, S], t2[:, S], OP.subtract)
                TT(Lp[:, r - 1, i, S], t1[:, S], rdiag[:, i - 1, S], OP.mult)
            # --- corner = sqrt(clip(1 - sum w^2, 1e-12)) ---
            PT(sacc[:, S], Lp[:, r - 1, 0, S], Lp[:, r - 1, 0, S], OP.mult)
            for i in range(1, r):
                PT(t2[:, S], Lp[:, r - 1, i, S], Lp[:, r - 1, i, S], OP.mult)
                PT(sacc[:, S], sacc[:, S], t2[:, S], OP.add)
            nc.vector.tensor_scalar(t1[:, S], sacc[:, S], -1.0, 1.0,
                                    op0=OP.mult, op1=OP.add)
            nc.vector.tensor_scalar(t1[:, S], t1[:, S], 1e-12, None, op0=OP.max)
            nc.scalar.sqrt(Lp[:, r - 1, r, S], t1[:, S])
            nc.vector.reciprocal(out=rdiag[:, r - 1, S], in_=Lp[:, r - 1, r, S])
            # --- forward solve collapsed: y_r = selval / L_rr ---
            TT(yv[:, r, S], yv[:, r, S], rdiag[:, r - 1, S], OP.mult)
            # --- backward solve L^T gam = y ---
            for i in range(k - 1, -1, -1):
                ts_copy(t1[:, S], yv[:, i, S])
                for q in range(i + 1, k):
                    PT(t2[:, S], Lp[:, q - 1, i, S], gam[:, q, S], OP.mult)
                    PT(t1[:, S], t1[:, S], t2[:, S], OP.subtract)
                if i == 0:
                    ts_copy(gam[:, 0, S], t1[:, S])
                else:
                    TT(gam[:, i, S], t1[:, S], rdiag[:, i - 1, S], OP.mult)
        if k < K:
            nc.vector.tensor_scalar(ngam[:, 0:k, S], gam[:, 0:k, S], -1.0,
                                    None, op0=OP.mult)

    # ---- main iteration loop, streams interleaved for pipelining ----
    for k in range(1, kmax + 1):
        for s in range(nstream):
            tiles = list(range(s * ts_per_s, (s + 1) * ts_per_s))
            S = slice(s * ts_per_s, (s + 1) * ts_per_s)
            for g0 in range(0, len(tiles), selgrp):
                grp = tiles[g0:g0 + selgrp]
                for t in grp:
                    emit_select(k, t)
                emit_mn8(grp)
                for t in grp:
                    emit_index(k, t)
                gS = slice(grp[0], grp[-1] + 1)
                nc.vector.tensor_reduce(idxs[:, k - 1, gS],
                                        idx8[:, gS, 0:2],
                                        axis=AX.X, op=OP.min)
                for t in grp:
                    emit_gather(k, t)
            emit_selval(k, S)
        for s in range(nstream):
            tiles = list(range(s * ts_per_s, (s + 1) * ts_per_s))
            S = slice(s * ts_per_s, (s + 1) * ts_per_s)
            for t in tiles:
                emit_dots(k, t)
            emit_solve(k, S)

    # ---- finalize: recon, outputs, loss ----
    for t in range(T if fin else 0):
        recT = smps.tile([P, P], F32, tag="sm")
        for j in range(K):
            asc = dgp.tile([P, C], F32)
            nc.scalar.mul(asc[:], A[:, j, t, :], gam[:, j, t:t + 1])
            nc.tensor.matmul(out=recT[:C, :], lhsT=asc[:], rhs=ident[:],
                             start=(j == 0), stop=(j == K - 1))
        errT = finp.tile([C, P], F32, tag="err")
        steT = finp.tile([C, P], F32, tag="ste")
        sqT = finp.tile([C, P], F32, tag="sq")
        TT(errT[:], recT[:C, :], x_all[:, tsl(t)], OP.subtract)
        nc.vector.scalar_tensor_tensor(
            out=sqT[:], in0=errT[:], scalar=1.0, in1=errT[:],
            op0=OP.mult, op1=OP.mult, accum_out=ssep[:, t:t + 1])
        TT(steT[:], x_all[:, tsl(t)], errT[:], OP.add)
        nc.sync.dma_start(io["zdl"][:, tsl(t)], steT[:])

    # support / coeffs: SBUF [P, K, T] -> DRAM [M, K] with m = t*128+p
    sup_ap = bass.AP(tensor=io["sup"].tensor, offset=0,
                     ap=[[K, P], [1, K], [K * P, T]])
    coe_ap = bass.AP(tensor=io["coe"].tensor, offset=0,
                     ap=[[K, P], [1, K], [K * P, T]])
    nc.sync.dma_start(sup_ap, idxs[:].bitcast(mybir.dt.int32))
    nc.sync.dma_start(coe_ap, gam[:])

    if not fin:
        nc.vector.memset(ssep[:], 0.0)
        zz = finp.tile([C, P], F32, tag="ste")
        nc.vector.memset(zz[:], 0.0)
        for t in range(T):
            nc.sync.dma_start(io["zdl"][:, tsl(t)], zz[:])
    # loss partial: sum over everything via ones-matmul
    nc.vector.tensor_reduce(ssetot[:], ssep[:], axis=AX.X, op=OP.add)
    lps = smps.tile([P, P], F32, tag="sm")
    nc.tensor.matmul(out=lps[:1, :1], lhsT=ssetot[:], rhs=ones64[:],
                     start=True, stop=True)
    nc.scalar.copy(sse_sb[:], lps[:1, :1])
    nc.sync.dma_start(io["sse"], sse_sb[:])


def build_nc(T: int = H * W // P, nstream: int = 2, kmax: int = K, fin: bool = True, selgrp: int = SELGRP, dots_pool_frac: int = 0):
    nc = bacc.Bacc("TRN2", target_bir_lowering=False, debug=False,
                   num_devices=NCORES)
    M = T * P
    io = dict(
        x=nc.dram_tensor("x", [C, M], F32, kind="ExternalInput").ap(),
        dn=nc.dram_tensor("dn", [C, N], F32, kind="ExternalInput").ap(),
        dnt=nc.dram_tensor("dnt", [N, C], F32, kind="ExternalInput").ap(),
        zdl=nc.dram_tensor("zdl", [C, M], F32, kind="ExternalOutput").ap(),
        sup=nc.dram_tensor("sup", [M, K], mybir.dt.int32,
                           kind="ExternalOutput").ap(),
        coe=nc.dram_tensor("coe", [M, K], F32, kind="ExternalOutput").ap(),
        sse=nc.dram_tensor("sse", [1, 1], F32, kind="ExternalOutput").ap(),
    )
    with tile.TileContext(nc) as tc:
        with ExitStack() as ctx:
            emit_omp(ctx, tc, io, T, nstream, kmax=kmax, fin=fin, selgrp=selgrp, dots_pool_frac=dots_pool_frac)
    nc.compile()
    return nc


_NC_CACHE: dict = {}


def _get_nc():
    if "nc" not in _NC_CACHE:
        _NC_CACHE["nc"] = build_nc()
    return _NC_CACHE["nc"]


def host_prep(dictionary: np.ndarray):
    """Normalize dictionary columns exactly like the reference (fp32)."""
    d = dictionary.astype(np.float32)
    norms = np.sqrt(np.sum(d * d, axis=0, dtype=np.float32),
                    dtype=np.float32).astype(np.float32)
    dn = d / np.maximum(norms, np.float32(EPS))[None, :]
    return np.ascontiguousarray(dn.astype(np.float32))


def kernel(z_e: np.ndarray, dictionary: np.ndarray, trace: bool = False):
    z_e = np.asarray(z_e, dtype=np.float32)
    dn = host_prep(np.asarray(dictionary))
    dnt = np.ascontiguousarray(dn.T)
    nc = _get_nc()
    in_maps = [
        {"x": np.ascontiguousarray(z_e[b].reshape(C, H * W)),
         "dn": dn, "dnt": dnt}
        for b in range(B)
    ]
    res = run_bass_kernel_spmd(nc, in_maps, core_ids=list(range(NCORES)),
                               trace=trace)
    outs = res.results
    z_dl = np.stack([outs[b]["zdl"].reshape(C, H, W) for b in range(B)])
    support = np.stack([outs[b]["sup"].reshape(H, W, K) for b in range(B)])
    coeffs = np.stack([outs[b]["coe"].reshape(H, W, K) for b in range(B)])
    sse = np.sum([outs[b]["sse"][0, 0] for b in range(B)], dtype=np.float32)
    nelem = np.float32(B * C * H * W)
    mse = np.float32(sse / nelem)
    loss = np.float32(mse + 0.25 * mse)
    if trace:
        return (z_dl, loss, support, coeffs), res
    return z_dl, loss, support, coeffs
